# revision 17
# baseline (speedup 1.0000x reference)
"""MultiHeadAttention + residual + LayerNorm Trainium2 kernel (8 NeuronCores).

Sharding: core c handles batch b = c//2 and query half h = c%2 (1024 queries).
Each core computes K/V projections for the full 2048-token sequence of its
batch (duplicated with its partner core; no cross-core communication at all),
Q projection for its local 1024 queries, attention, output projection,
residual add and LayerNorm for its local queries.

Everything on-chip is kept "transposed" (feature dim on partitions, tokens on
the free dim) so that no transposes are ever needed:
  - x^T, xq^T are passed in pre-transposed by the host.
  - K^T = w_k @ x^T          (lhsT = w_k^T passed pre-transposed)
  - V   = x @ w_v^T          (lhsT = x^T tiles, natural [token, dv] layout)
  - S^T[keys, q] = K Q^T     (lhsT = K^T tile, rhs = Q^T tile)
  - P^T = exp(SCALE * S^T)   (ScalarE, fused scale; scores are small enough
                              that softmax needs no max subtraction)
  - C~^T[d, q] = V_ext^T P^T (lhsT = V_ext = [V | 1]; row 64 of the result is
                              the softmax denominator - free on the PE)
  - ctx^T = C~^T[0:64] * (1/denom)   (denom broadcast across partitions via a
                                      rank-1 ones matmul)
  - y^T = w_o @ ctx^T + b_o + xq^T, then LayerNorm over the partition dim via
    ones-matmul statistics and rank-1 broadcast matmuls.
Biases everywhere are folded into the matmul accumulations as rank-1 updates.
Matmuls run in float32r (full PE rate at N=512); P^T / V_ext use bf16.
"""

import os
from contextlib import ExitStack

import numpy as np

import concourse.bass as bass
import concourse.mybir as mybir
import concourse.tile as tile

B, S, D, H, DH = 4, 2048, 512, 8, 64
SQ = S // 2          # local queries per core
NCORES = 8
P = 128
NC_D = D // P        # 4 chunks of the feature dim
NC_S = S // P        # 16 key chunks
NQB = SQ // 512      # 2 query blocks of 512
SCALE = float(1.0 / np.sqrt(np.float32(D)))
EPS = 1e-5

F32 = mybir.dt.float32
F32R = mybir.dt.float32r
BF16 = mybir.dt.bfloat16
ALU = mybir.AluOpType
AFT = mybir.ActivationFunctionType


def _r(ap):
    """Matmul operands are declared float32r; nothing to do."""
    return ap


def _split_multiwait_json(bir, cap=1):
    """The walrus build here encodes at most one sync-wait command per
    instruction (self-loading f32r matmuls and drains with 2+ waits fail
    codegen with 'Too many sync wait commands'). Hoist excess waits onto
    preceding single-wait NoOps on the same engine - engine streams execute
    in order, so waiting earlier is always safe."""
    n = 0
    for fn in bir.get("functions", []):
        for bb in fn.get("blocks", []):
            out = []
            for ins in bb.get("instructions", []):
                si = ins.get("sync_info")
                waits = (si or {}).get("on_wait") or []
                if len(waits) > cap:
                    extra, si["on_wait"] = waits[:-cap], waits[-cap:]
                    for i in range(0, len(extra), cap):
                        n += 1
                        out.append(
                            {
                                "debug": ins.get("debug", 0),
                                "engine": ins["engine"],
                                "ins": [],
                                "outs": [],
                                "name": f"{ins['name']}-wsplit{n}",
                                "opcode": "NoOp",
                                "sync_info": {
                                    "on_wait": extra[i : i + cap],
                                    "on_update": [],
                                },
                            }
                        )
                out.append(ins)
            bb["instructions"] = out
    return bir


def _patch_serialization(nc):
    import orjson

    orig = nc.to_json_bytes

    def to_json_bytes_split():
        return orjson.dumps(_split_multiwait_json(orjson.loads(orig())))

    nc.to_json_bytes = to_json_bytes_split
    return nc


def build_nc():
    nc = bass.Bass("TRN2", target_bir_lowering=False)

    xt_d = nc.dram_tensor("xt", [D, S], F32R, kind="ExternalInput")
    xqt_d = nc.dram_tensor("xqt", [D, SQ], F32R, kind="ExternalInput")
    wqt_d = nc.dram_tensor("wqt", [D, D], F32R, kind="ExternalInput")
    wkt_d = nc.dram_tensor("wkt", [D, D], F32R, kind="ExternalInput")
    wvt_d = nc.dram_tensor("wvt", [D, D], F32R, kind="ExternalInput")
    wot_d = nc.dram_tensor("wot", [D, D], F32R, kind="ExternalInput")
    bq_d = nc.dram_tensor("bq", [D], F32R, kind="ExternalInput")
    bk_d = nc.dram_tensor("bk", [D], F32R, kind="ExternalInput")
    bv_d = nc.dram_tensor("bv", [D], F32R, kind="ExternalInput")
    bo_d = nc.dram_tensor("bo", [D], F32R, kind="ExternalInput")
    gamma_d = nc.dram_tensor("gamma", [D], F32, kind="ExternalInput")
    beta_d = nc.dram_tensor("beta", [D], F32, kind="ExternalInput")
    ytd = nc.dram_tensor("ytd", [D, SQ], F32, kind="ExternalOutput")

    with (
        tile.TileContext(nc) as tc,
        ExitStack() as ctx,
        nc.allow_low_precision(reason="float32r outputs feed full-rate PE matmuls"),
    ):
        singles = ctx.enter_context(tc.tile_pool(name="singles", bufs=1))
        wpool = ctx.enter_context(tc.tile_pool(name="wpool", bufs=2))
        ptpool = ctx.enter_context(tc.tile_pool(name="ptpool", bufs=3))
        ytpool = ctx.enter_context(tc.tile_pool(name="ytpool", bufs=1))
        rows = ctx.enter_context(tc.tile_pool(name="rows", bufs=2))
        ps_sc = ctx.enter_context(tc.tile_pool(name="ps_sc", bufs=2, space="PSUM"))
        ps_ct = ctx.enter_context(tc.tile_pool(name="ps_ct", bufs=2, space="PSUM"))
        ps_pj = ctx.enter_context(tc.tile_pool(name="ps_pj", bufs=2, space="PSUM"))

        # ---- persistent SBUF tensors ----
        xt = singles.tile([P, NC_D, S], F32R)       # x^T  [din, token]
        xqt = singles.tile([P, NC_D, SQ], F32R)     # local x^T
        kt = singles.tile([P, NC_D, S], F32R)       # K^T  [dk, token]
        qt = singles.tile([P, NC_D, SQ], F32R)      # Q^T  [dq, local token]
        vext = singles.tile([P, NC_S, H, DH + 1], BF16)  # [token, head, dv|1]
        ctxt = singles.tile([P, NC_D, SQ], F32R)    # ctx^T [din, local token]

        nc.gpsimd.dma_start(xt[:], xt_d[:, :].rearrange("(c p) t -> p c t", p=P))
        nc.gpsimd.dma_start(xqt[:], xqt_d[:, :].rearrange("(c p) t -> p c t", p=P))

        # bias rows on partition 0 (used as rank-1 matmul operands)
        bias_rows = {}
        for name, dten in (("bq", bq_d), ("bk", bk_d), ("bv", bv_d), ("bo", bo_d)):
            row = singles.tile([1, D], F32R, tag=f"row_{name}")
            nc.gpsimd.dma_start(row[:], dten[:][None, :])
            bias_rows[name] = row
        neg_gamma = singles.tile([1, D], F32R)
        gamma_row = singles.tile([1, D], F32)
        nc.gpsimd.dma_start(gamma_row[:], gamma_d[:][None, :])
        nc.vector.tensor_scalar_mul(neg_gamma[:], gamma_row[:], -1.0)
        gamma_col = singles.tile([P, NC_D], F32)
        beta_col = singles.tile([P, NC_D], F32)
        nc.gpsimd.dma_start(gamma_col[:], gamma_d[:].rearrange("(c p) -> p c", p=P))
        nc.gpsimd.dma_start(beta_col[:], beta_d[:].rearrange("(c p) -> p c", p=P))

        ones_row = singles.tile([1, 512], F32R)     # rank-1 rhs
        ones_col = singles.tile([1, P], F32R)       # rank-1 lhsT (M=128)
        ones_p = singles.tile([P, 1], F32R)         # stats lhsT (contract 128)
        ones_f32 = singles.tile([P, 512], F32)
        eps_tile = singles.tile([1, 1], F32)
        nc.vector.memset(ones_f32[:], 1.0)
        nc.vector.tensor_copy(ones_row[:], ones_f32[0:1, :])
        nc.vector.tensor_copy(ones_col[:], ones_f32[0:1, 0:P])
        nc.vector.tensor_copy(ones_p[:], ones_f32[:, 0:1])
        nc.vector.memset(eps_tile[:], EPS)
        # fill all of vext with 1.0; the V-projection copies overwrite
        # columns 0..DH-1 per head, leaving the ones column at DH
        nc.vector.memset(vext[:], 1.0)

        # ---- phase 2: projections (all contract over din in chunks of 128) --
        def load_w(dten, name):
            w = wpool.tile([P, NC_D, D], F32R, tag="w")
            nc.gpsimd.dma_start(w[:], dten[:, :].rearrange("(c p) f -> p c f", p=P))
            return w

        wk = load_w(wkt_d, "wk")
        # K^T[dk, t] = sum_c wkt[c, dk]^T xt[c, t] + bk x 1^T
        for m in range(NC_D):
            for nb in range(S // 512):
                ps = ps_pj.tile([P, 512], F32, tag="pj")
                for c in range(NC_D):
                    nc.tensor.matmul(
                        ps[:],
                        _r(wk[:, c, m * P : (m + 1) * P]),
                        _r(xt[:, c, nb * 512 : (nb + 1) * 512]),
                        start=(c == 0),
                        stop=False,
                    )
                nc.tensor.matmul(
                    ps[:],
                    _r(bias_rows["bk"][0:1, m * P : (m + 1) * P]),
                    _r(ones_row[0:1, :]),
                    start=False,
                    stop=True,
                )
                nc.vector.tensor_copy(kt[:, m, nb * 512 : (nb + 1) * 512], ps[:])

        wv = load_w(wvt_d, "wv")
        # V[t, dv] = sum_c xt[c, t]^T wvt[c, dv] + 1 x bv^T  -> vext[:, t, :, 0:64]
        for t in range(NC_S):
            ps = ps_pj.tile([P, 512], F32, tag="pj")
            for c in range(NC_D):
                nc.tensor.matmul(
                    ps[:],
                    _r(xt[:, c, t * P : (t + 1) * P]),
                    _r(wv[:, c, :]),
                    start=(c == 0),
                    stop=False,
                )
            nc.tensor.matmul(
                ps[:],
                _r(ones_col[0:1, :]),
                _r(bias_rows["bv"][0:1, :]),
                start=False,
                stop=True,
            )
            nc.vector.tensor_copy(
                vext[:, t, :, 0:DH],
                ps[:].rearrange("p (h d) -> p h d", h=H),
            )

        wq = load_w(wqt_d, "wq")
        # Q^T[dq, t_local] like K^T but against xqt
        for m in range(NC_D):
            for nb in range(NQB):
                ps = ps_pj.tile([P, 512], F32, tag="pj")
                for c in range(NC_D):
                    nc.tensor.matmul(
                        ps[:],
                        _r(wq[:, c, m * P : (m + 1) * P]),
                        _r(xqt[:, c, nb * 512 : (nb + 1) * 512]),
                        start=(c == 0),
                        stop=False,
                    )
                nc.tensor.matmul(
                    ps[:],
                    _r(bias_rows["bq"][0:1, m * P : (m + 1) * P]),
                    _r(ones_row[0:1, :]),
                    start=False,
                    stop=True,
                )
                nc.vector.tensor_copy(qt[:, m, nb * 512 : (nb + 1) * 512], ps[:])

        # ---- phase 3: attention, head pairs (rows 0:64 / 64:128 run
        # concurrently on the PE via row tiling) ----
        for pair in range(H // 2):
            for qb in range(NQB):
                qs = slice(qb * 512, (qb + 1) * 512)
                cts = [
                    ps_ct.tile([P, 512], F32, tag="ct", name=f"ct{i}")
                    for i in range(2)
                ]
                for kc in range(NC_S):
                    sc = ps_sc.tile([P, 2, 512], F32, tag="sc")
                    for hh in range(2):
                        rs = slice(hh * DH, (hh + 1) * DH)
                        nc.tensor.matmul(
                            sc[:, hh, :],
                            _r(kt[rs, pair, kc * P : (kc + 1) * P]),
                            _r(qt[rs, pair, qs]),
                            start=True,
                            stop=True,
                        )
                    pt = ptpool.tile([P, 2, 512], BF16, tag="pt")
                    nc.scalar.activation(pt[:], sc[:], AFT.Exp, scale=SCALE)
                    for hh in range(2):
                        nc.tensor.matmul(
                            cts[hh][0 : DH + 1, :],
                            vext[:, kc, 2 * pair + hh, :],
                            pt[:, hh, :],
                            start=(kc == 0),
                            stop=(kc == NC_S - 1),
                        )
                # normalize: ctx^T = C~[0:64] / denom (denom = row 64)
                for hh in range(2):
                    rec = rows.tile([1, 512], F32R, tag="rec")
                    nc.vector.reciprocal(rec[:], cts[hh][DH : DH + 1, :])
                    rb = ps_pj.tile([P, 512], F32, tag="pj")
                    nc.tensor.matmul(
                        rb[0:DH, :],
                        _r(ones_col[0:1, 0:DH]),
                        _r(rec[0:1, :]),
                        start=True,
                        stop=True,
                    )
                    cslice = ctxt[hh * DH : (hh + 1) * DH, pair, qs]
                    nc.vector.tensor_copy(cslice, cts[hh][0:DH, :])
                    nc.vector.tensor_tensor(cslice, cslice, rb[0:DH, :], ALU.mult)

        # ---- phase 4: output projection + residual + LayerNorm ----
        wo = load_w(wot_d, "wo")
        inv_d = 1.0 / D
        for qb in range(NQB):
            qs = slice(qb * 512, (qb + 1) * 512)
            yt = ytpool.tile([P, NC_D, 512], F32R, tag="yt")
            for m in range(NC_D):
                ps = ps_pj.tile([P, 512], F32, tag="pj")
                for c in range(NC_D):
                    nc.tensor.matmul(
                        ps[:],
                        _r(wo[:, c, m * P : (m + 1) * P]),
                        _r(ctxt[:, c, qs]),
                        start=(c == 0),
                        stop=False,
                    )
                nc.tensor.matmul(
                    ps[:],
                    _r(bias_rows["bo"][0:1, m * P : (m + 1) * P]),
                    _r(ones_row[0:1, :]),
                    start=False,
                    stop=True,
                )
                # residual
                nc.vector.tensor_tensor(yt[:, m, :], ps[:], xqt[:, m, qs], ALU.add)

            # stats over the feature (partition) dim via ones-matmuls
            mean_ps = ps_ct.tile([P, 512], F32, tag="ct")
            msq_ps = ps_ct.tile([P, 512], F32, tag="ct")
            for m in range(NC_D):
                nc.tensor.matmul(
                    mean_ps[0:1, :],
                    _r(ones_p[:, 0:1]),
                    _r(yt[:, m, :]),
                    start=(m == 0),
                    stop=(m == NC_D - 1),
                )
            for m in range(NC_D):
                sq = ptpool.tile([P, 512], F32R, tag="pt")
                nc.vector.tensor_tensor(sq[:], yt[:, m, :], yt[:, m, :], ALU.mult)
                nc.tensor.matmul(
                    msq_ps[0:1, :],
                    _r(ones_p[:, 0:1]),
                    _r(sq[:]),
                    start=(m == 0),
                    stop=(m == NC_D - 1),
                )
            mu = rows.tile([1, 512], F32, tag="mu")
            msq = rows.tile([1, 512], F32, tag="msq")
            rstd = rows.tile([1, 512], F32R, tag="rstd")
            mur = rows.tile([1, 512], F32R, tag="mur")
            nc.vector.tensor_scalar_mul(mu[:], mean_ps[0:1, :], inv_d)
            nc.vector.tensor_scalar_mul(msq[:], msq_ps[0:1, :], inv_d)
            # var = msq - mu^2 (into msq)
            musq = rows.tile([1, 512], F32, tag="musq")
            nc.vector.tensor_tensor(musq[:], mu[:], mu[:], ALU.mult)
            nc.vector.tensor_tensor(msq[:], msq[:], musq[:], ALU.subtract)
            nc.scalar.activation(rstd[:], msq[:], AFT.Sqrt, bias=eps_tile[0:1, :])
            nc.vector.reciprocal(rstd[:], rstd[:])
            nc.vector.tensor_tensor(mur[:], mu[:], rstd[:], ALU.mult)
            # broadcast rstd and (-gamma x mu*rstd) via rank-1 matmuls
            sb = ps_sc.tile([P, 512], F32, tag="sc")
            nc.tensor.matmul(
                sb[:], _r(ones_col[0:1, :]), _r(rstd[0:1, :]), start=True, stop=True
            )
            for m in range(NC_D):
                tb = ps_sc.tile([P, 512], F32, tag="sc")
                nc.tensor.matmul(
                    tb[:],
                    _r(neg_gamma[0:1, m * P : (m + 1) * P]),
                    _r(mur[0:1, :]),
                    start=True,
                    stop=True,
                )
                fin = ptpool.tile([P, 512], F32, tag="pt")
                # fin = (y * gamma) * rstd_b
                nc.vector.scalar_tensor_tensor(
                    fin[:],
                    yt[:, m, :],
                    gamma_col[:, m : m + 1],
                    sb[:],
                    ALU.mult,
                    ALU.mult,
                )
                # fin = (fin + beta) + (-gamma * mu * rstd)_b
                nc.vector.scalar_tensor_tensor(
                    fin[:],
                    fin[:],
                    beta_col[:, m : m + 1],
                    tb[:],
                    ALU.add,
                    ALU.add,
                )
                nc.sync.dma_start(
                    ytd[:, :].rearrange("(c p) t -> p c t", p=P)[:, m, qs],
                    fin[:],
                )

    return _patch_serialization(nc)


_nc_cache = None


def _get_nc():
    global _nc_cache
    if _nc_cache is None:
        _nc_cache = build_nc()
    return _nc_cache


def make_in_maps(x, w_q, b_q, w_k, b_k, w_v, b_v, w_o, b_o, ln_gamma, ln_beta):
    f = lambda a: np.ascontiguousarray(np.asarray(a), dtype=np.float32)
    shared = dict(
        wqt=f(np.asarray(w_q).T), wkt=f(np.asarray(w_k).T),
        wvt=f(np.asarray(w_v).T), wot=f(np.asarray(w_o).T),
        bq=f(b_q), bk=f(b_k), bv=f(b_v), bo=f(b_o),
        gamma=f(ln_gamma), beta=f(ln_beta),
    )
    x = f(x)
    in_maps = []
    for c in range(NCORES):
        b, half = divmod(c, 2)
        off = half * SQ
        in_maps.append(
            dict(
                xt=np.ascontiguousarray(x[b].T),
                xqt=np.ascontiguousarray(x[b, off : off + SQ].T),
                **shared,
            )
        )
    return in_maps


def assemble(results):
    y = np.empty((B, S, D), np.float32)
    for c in range(NCORES):
        b, half = divmod(c, 2)
        off = half * SQ
        y[b, off : off + SQ, :] = np.ascontiguousarray(results[c]["ytd"].T)
    return y


def run(inputs, trace=False, **kwargs):
    from concourse.bass_utils import run_bass_kernel_spmd

    nc = _get_nc()
    in_maps = make_in_maps(**inputs)
    res = run_bass_kernel_spmd(
        nc, in_maps, core_ids=list(range(NCORES)), trace=trace, **kwargs
    )
    return assemble(res.results), res


def kernel(**inputs):
    y, _ = run(inputs, trace=False)
    return y
